# revision 22
# baseline (speedup 1.0000x reference)
"""Trainium2 Bass kernel for nn_Block_31954556682442 (spiking MoE-SSA block).

Sharding: pure data-parallel over batch B=8 -> one sample (4 LIF time steps)
per NeuronCore, zero collectives. v3 design:
  - ALL weight matmuls single-term bf16 (W and x both bf16-rounded); CPU-sim
    rel err 4.4e-3 vs the 2e-2 gate
  - kv-first attention: res_e = q_e^T (k v^T); kv integers <=256 exact bf16;
    k^T computed in v's layout by widening the v matmul rhs (no transposes)
  - attention-path LIF state kept in bf16 (values quantized to 0.5 and
    compare-safe), enabling 2x/4x DVE modes
  - depthwise 3x3 conv moved to the PE: per-(ch,tap) diagonal bf16 weights
    x zero-padded spike tiles; bias via diag @ ones
  - fc2 split: t-pair 0 accumulated across the ch loop (3 PSUM banks),
    t-pair 1 as a tail overlapped with the final LIF
  - LIF scans in 2^t-scaled form as in v2
Self-contained: hardcodes all shapes; no sibling imports.
"""
import numpy as np
import ml_dtypes

import concourse.bacc as bacc
import concourse.mybir as mybir
import concourse.tile as tile
from concourse.bass_utils import run_bass_kernel_spmd

F32 = mybir.dt.float32
BF16 = mybir.dt.bfloat16
AL = mybir.AluOpType
AF = mybir.ActivationFunctionType

T, B, C, N = 4, 8, 384, 256
ED = 96
NE = 4
HID, HH = 2048, 1024
S = float(1.0 / np.sqrt(1.0 + 1e-5))
P = 128


def _body(nc, tc, d):
    from contextlib import ExitStack
    VE = nc.vector
    GE = nc.gpsimd

    with ExitStack() as ctx:
        def pool(name, bufs, space="SBUF"):
            return ctx.enter_context(tc.tile_pool(name=name, bufs=bufs, space=space))

        wp = pool("wp", 1)
        mp = pool("mp", 1)
        xs_p = pool("xs_p", 3)       # (128,1024) f32 x, doubles as x_new
        xq_p = pool("xq_p", 3)       # (128,1024) bf16 rounded x
        xqt_p = pool("xqt_p", 3)     # (96,1024) f32 q pre-act
        xvt_p = pool("xvt_p", 3)     # (128,960) f32 v|k pre-act
        xrt_p = pool("xrt_p", 2)     # (128,8)
        spq_p = pool("spq_p", 4)     # (96,1024) bf16 q spikes
        vks_p = pool("vks_p", 4)     # (128,960) bf16 v|k spikes
        wsp_p = pool("wsp_p", 4)     # (128,8) f32 router spikes
        kv_p = pool("kv_p", 2)       # (96,384) bf16
        xr_p = pool("xr_p", 4)       # (128,768) bf16 res pre-act
        rs_p = pool("rs_p", 2)       # (128,768) bf16 masked res spikes
        y_p = pool("y_p", 4)         # (128,768) bf16
        ydn_p = pool("ydn_p", 3)     # (128,1024) bf16
        xp_p = pool("xp_p", 2)       # (128,768) f32 proj pre-act
        xh_p = pool("xh_p", 3)       # (128,2048) f32
        sp2_p = pool("sp2_p", 3)     # (128,1024) bf16 gate spikes
        xp1_p = pool("xp1_p", 1)     # padded dw spikes, 2 tiles
        acc_p = pool("acc_p", 3)     # (128,1024) f32 dw-conv acc
        mg0_p = pool("mg0_p", 2)     # (128,512) bf16 t-pair0 gated spikes
        mg1_p = pool("mg1_p", 8)     # (128,512) bf16 t-pair1 gated spikes
        mh_p = pool("mh_p", 2)       # (128,512) f32
        mdw_p = pool("mdw_p", 2)     # (128,256) f32
        xo_p = pool("xo_p", 2)       # (128,768) bf16 fc2 pre-act
        of_p = pool("of_p", 2)       # (128,768) f32 final out

        psum_ad = ExitStack()
        ps_ad = psum_ad.enter_context(
            tc.tile_pool(name="ps_ad", bufs=6, space="PSUM"))

        # ---------------- weight loads ----------------
        def wload(name, shape, dt=F32, src=None):
            w = wp.tile(shape, dt, name=name, tag=name)
            nc.sync.dma_start(out=w, in_=d[name] if src is None else src)
            return w

        ident = wload('ident', [P, P], BF16)
        # PE warmup: dummy matmuls to flip HAM to K=8/8 before phase A
        pwarm = ps_ad.tile([P, P], F32, name="pwarm", tag="pm")
        for wi in range(12):
            nc.tensor.matmul(pwarm, ident, ident, start=True, stop=True)
        warm_sink = wp.tile([P, 1], F32, name="warm_sink", tag="warm_sink")
        nc.scalar.activation(warm_sink, pwarm[:, 0:1], AF.Copy)

        # xs first (phase A starts on these)
        xs_kt = []
        for kt in range(3):
            x_ = xs_p.tile([P, 4 * N], F32, name=f"xs{kt}", tag="t")
            xs_kt.append(x_)
        for kt in range(3):
            nc.sync.dma_start(out=xs_kt[kt], in_=d['xin'][kt*P:(kt+1)*P, :])
        q_w, vk_w, pj_w, f1_w, f2_w = [], [], [], [], []
        for kt in range(3):
            q_w.append(wload(f'q_w{kt}', [P, 384], BF16, d['q_w'][kt*P:(kt+1)*P, :]))
            vk_w.append(wload(f'vk_w{kt}', [P, 484], BF16, d['vk_w'][kt*P:(kt+1)*P, :]))
        sq = wload('s_q', [ED, 32])
        a_q, b_q = sq[:, 0:16], sq[:, 16:32]
        rb2 = wload('rb2', [1, 484], BF16); ones = wload('ones', [1, P], BF16)
        for kt in range(3):
            pj_w.append(wload(f'pj_w{kt}', [P, 384], BF16, d['pj_w'][kt*P:(kt+1)*P, :]))
        spo = wload('s_po', [P, 48])
        a_p, b_p = spo[:, 0:12], spo[:, 12:24]
        a_o, b_o = spo[:, 24:36], spo[:, 36:48]
        for kt in range(3):
            f1_w.append(wload(f'f1_w{kt}', [P, 2048], BF16, d['f1_w'][kt*P:(kt+1)*P, :]))
        sh = wload('s_h', [P, 128])
        a_h, b_h = sh[:, 0:64], sh[:, 64:128]
        dwd = wload('dwd', [P, 72 * P], BF16)
        bdw = wload('b_dw', [P, 8])
        for ch in range(8):
            f2_w.append(wload(f'f2_w{ch}', [P, 384], BF16, d['f2_w'][ch*P:(ch+1)*P, :]))

        # ---------------- x -> bf16 ----------------
        xq_kt = []
        for kt in range(3):
            q_ = xq_p.tile([P, 4 * N], BF16, name=f"xq{kt}", tag="t")
            nc.scalar.activation(q_, xs_kt[kt], AF.Copy)
            xq_kt.append(q_)

        # ---------------- phase A: q / v|k / router matmuls + evicts ----------------
        m_kq = mp.tile([ED, 4 * N], BF16, name="m_kq", tag="m_kq")
        m_vt = mp.tile([P, 968], BF16, name="m_vt", tag="m_vt")
        m_p = mp.tile([P, 768], BF16, name="m_p", tag="m_p")
        m_o = [mp.tile([P, N], BF16, name=f"m_o{i}", tag=f"m_o{i}") for i in range(3)]

        xq_t = [xqt_p.tile([ED, 4 * N], BF16, name=f"xqt{t}", tag="t") for t in range(T)]
        xvt_t = [xvt_p.tile([P, 968], BF16, name=f"xvt{t}", tag="t") for t in range(T)]

        for tp in range(2):
            for u in range(NE):
                pt = ps_ad.tile([ED, 512], F32, name=f"pq{u}_{tp}", tag="pm")
                for kt in range(3):
                    nc.tensor.matmul(pt, q_w[kt][:, 96*u:96*(u+1)],
                                     xq_kt[kt][:, tp*512:(tp+1)*512],
                                     start=(kt == 0), stop=(kt == 2))
                for ti in range(2):
                    t = tp * 2 + ti
                    c = u * 4 + t
                    nc.scalar.activation(xq_t[t][:, u*N:(u+1)*N], pt[:, ti*N:(ti+1)*N],
                                         AF.Identity, bias=b_q[:, c:c+1], scale=a_q[:, c:c+1])
        for t in range(T):
            for mt in range(2):
                pv = ps_ad.tile([P, 484], F32, name=f"pvt{t}_{mt}", tag="pm")
                for kt in range(3):
                    nc.tensor.matmul(pv, xq_kt[kt][:, t*N + mt*P: t*N + (mt+1)*P],
                                     vk_w[kt], start=(kt == 0), stop=False,
                                     skip_group_check=True)
                nc.tensor.matmul(pv[:, 480:484], ones, rb2[:, 480:484],
                                 start=False, stop=True, skip_group_check=True)
                nc.scalar.activation(xvt_t[t][:, mt*484:(mt+1)*484], pv, AF.Copy,
                                     bias=0.0, scale=0.5 * float(2.0 ** t))

        # ---------------- phase B: LIF scans for q / v|k / r ----------------
        sp_q, vk_sp, w_sp = [], [], []
        for t in range(T):
            thr = float(2.0 ** t)
            U = xq_t[t]
            if t > 0:
                VE.tensor_add(U, m_kq, U)
            sp = spq_p.tile([ED, 4 * N], BF16, name=f"spq{t}", tag="t")
            VE.tensor_single_scalar(sp, U, thr, AL.is_ge)
            if t < T - 1:
                VE.scalar_tensor_tensor(out=m_kq, in0=U, scalar=thr, in1=U,
                                        op0=AL.is_lt, op1=AL.mult)
            sp_q.append(sp)

            U = xvt_t[t]
            if t > 0:
                VE.tensor_add(U, m_vt, U)
            vs = vks_p.tile([P, 968], BF16, name=f"vks{t}", tag="t")
            VE.tensor_single_scalar(vs, U, thr, AL.is_ge)
            # router spike columns as f32 (tensor_scalar scalar2 operand)
            ws = wsp_p.tile([P, 8], F32, name=f"wsp{t}", tag="t")
            U3 = U.rearrange("p (m c) -> p m c", m=2)
            VE.tensor_single_scalar(ws.rearrange("p (m c) -> p m c", m=2),
                                    U3[:, :, 480:484], thr, AL.is_ge)
            w_sp.append(ws)
            if t < T - 1:
                VE.scalar_tensor_tensor(out=m_vt, in0=U, scalar=thr, in1=U,
                                        op0=AL.is_lt, op1=AL.mult)
            vk_sp.append(vs)
            # keep-warm: tiny matmul chained on this step's spikes so the PE
            # HAM window never sees a fully idle interval during the LIF scan
            nc.tensor.matmul(pwarm[:, 0:1], ident, vs[:, 0:1], start=True, stop=True)

        # ---------------- phase C: kv-first experts ----------------
        # masked spikes: w_e*(U>=thr) via one tensor_scalar (4x mode);
        # y accumulated with plain bf16 tensor_tensor adds (2x mode)
        y = [None] * T
        m_res_e = [mp.tile([P, 768], BF16, name=f"m_res{e}", tag=f"m_res{e}")
                   for e in range(NE)]
        for t in range(T):
            thr = float(2.0 ** t)
            pkv = ps_ad.tile([ED, 384], F32, name=f"pkv{t}", tag="pm")
            for mt in range(2):
                nc.tensor.matmul(pkv, vk_sp[t][:, mt*484+384:mt*484+480],
                                 vk_sp[t][:, mt*484:mt*484+384],
                                 start=(mt == 0), stop=(mt == 1))
            kv = kv_p.tile([ED, 384], BF16, name=f"kv{t}", tag="t")
            nc.scalar.activation(kv, pkv, AF.Copy, bias=0.0, scale=0.5 * thr)
            yt = y_p.tile([P, 768], BF16, name=f"y{t}", tag="t")
            y[t] = yt
            for e in range(NE):
                m_res = m_res_e[e]
                xr = xr_p.tile([P, 768], BF16, name=f"xres{e}{t}", tag="t")
                for mt in range(2):
                    pr_ = ps_ad.tile([P, 384], F32, name=f"pres{e}{t}{mt}", tag="pm")
                    nc.tensor.matmul(pr_, sp_q[t][:, e*N + mt*P: e*N + (mt+1)*P],
                                     kv, start=True, stop=True)
                    nc.scalar.activation(xr[:, mt*384:(mt+1)*384], pr_, AF.Copy)
                U = xr
                if t > 0:
                    VE.tensor_add(U, m_res, U)
                dst = yt if e == 0 else rs_p.tile([P, 768], BF16,
                                                  name=f"rsm{e}{t}", tag="t")
                for mt in range(2):
                    VE.tensor_scalar(out=dst[:, mt*384:(mt+1)*384],
                                     in0=U[:, mt*384:(mt+1)*384],
                                     scalar1=thr,
                                     scalar2=w_sp[t][:, mt*4+e:mt*4+e+1],
                                     op0=AL.is_ge, op1=AL.mult)
                if t < T - 1:
                    VE.scalar_tensor_tensor(out=m_res, in0=U, scalar=thr, in1=U,
                                            op0=AL.is_lt, op1=AL.mult)
                if e > 0:
                    VE.tensor_add(yt, yt, dst)
                nc.tensor.matmul(pwarm[:, 0:1], ident, U[:, 0:1], start=True, stop=True)

        # ---------------- phase D: transpose y, proj, LIF, residual ----------------
        ydn = [ydn_p.tile([P, 4 * N], BF16, name=f"ydn{dt}", tag="t") for dt in range(3)]
        xp_t = [xp_p.tile([P, 768], BF16, name=f"xp{t}", tag="t") for t in range(T)]
        for tp in range(2):
            for t in (tp * 2, tp * 2 + 1):
                for mt in range(2):
                    for dt in range(3):
                        ptr = ps_ad.tile([P, P], BF16, name=f"ptr{t}{mt}{dt}", tag="pm")
                        nc.tensor.transpose(
                            ptr, y[t][:, mt*384 + dt*P: mt*384 + (dt+1)*P], ident)
                        nc.scalar.activation(ydn[dt][:, t*N + mt*P: t*N + (mt+1)*P],
                                             ptr, AF.Copy)
            for mt in range(3):
                pp = ps_ad.tile([P, 512], F32, name=f"pp{mt}_{tp}", tag="pm")
                for kt in range(3):
                    nc.tensor.matmul(pp, pj_w[kt][:, mt*P:(mt+1)*P],
                                     ydn[kt][:, tp*512:(tp+1)*512],
                                     start=(kt == 0), stop=(kt == 2))
                for ti in range(2):
                    t = tp * 2 + ti
                    c = mt * 4 + t
                    nc.scalar.activation(xp_t[t][:, mt*N:(mt+1)*N], pp[:, ti*N:(ti+1)*N],
                                         AF.Identity, bias=b_p[:, c:c+1], scale=a_p[:, c:c+1])
            for t in (tp * 2, tp * 2 + 1):
                thr = float(2.0 ** t)
                U = xp_t[t]
                if t > 0:
                    VE.tensor_add(U, m_p, U)
                if t < T - 1:
                    VE.scalar_tensor_tensor(out=m_p, in0=U, scalar=thr, in1=U,
                                            op0=AL.is_lt, op1=AL.mult)
                for mt in range(3):
                    # x_new overwrites xs in place (residual add)
                    VE.scalar_tensor_tensor(
                        out=xs_kt[mt][:, t*N:(t+1)*N], in0=U[:, mt*N:(mt+1)*N],
                        scalar=thr, in1=xs_kt[mt][:, t*N:(t+1)*N],
                        op0=AL.is_ge, op1=AL.add)
            # x_new -> bf16 for this time-pair (fc1 can start on tp=0)
            for kt in range(3):
                nc.scalar.activation(xq_kt[kt][:, tp*512:(tp+1)*512],
                                     xs_kt[kt][:, tp*512:(tp+1)*512], AF.Copy)

        # ---------------- phase E: MLP ----------------
        psum_ad.close()
        ps_e = ctx.enter_context(tc.tile_pool(name="ps_e", bufs=3, space="PSUM"))
        ps_dw = ctx.enter_context(tc.tile_pool(name="ps_dw", bufs=2, space="PSUM"))
        ps_po = ctx.enter_context(tc.tile_pool(name="ps_po", bufs=3, space="PSUM"))
        # padded dw-spike tiles (zero borders written once)
        xp1_bufs = [xp1_p.tile([P, 1296], BF16, name=f"xp1_{i}", tag=f"xp1_{i}")
                    for i in range(2)]
        for b_ in xp1_bufs:
            GE.memset(b_, 0.0)
        po1 = [ps_po.tile([P, 512], F32, name=f"po1_{mt}", tag="po")
               for mt in range(3)]
        TAPS = [(dy, dx) for dy in range(3) for dx in range(3)]
        mg_t = []
        for ch in range(8):
            xh = xh_p.tile([P, 2048], BF16, name=f"xh{ch}", tag="t")
            for half in range(2):
                mth = ch + 8 * half
                for tp in range(2):
                    ph = ps_e.tile([P, 512], F32, name=f"ph{ch}{half}{tp}", tag="pm")
                    for kt in range(3):
                        nc.tensor.matmul(ph, f1_w[kt][:, mth*P:(mth+1)*P],
                                         xq_kt[kt][:, tp*512:(tp+1)*512],
                                         start=(kt == 0), stop=(kt == 2))
                    for ti in range(2):
                        t = tp * 2 + ti
                        c = mth * 4 + t
                        nc.scalar.activation(
                            xh[:, half*1024 + t*N: half*1024 + (t+1)*N],
                            ph[:, ti*N:(ti+1)*N], AF.Identity,
                            bias=b_h[:, c:c+1], scale=a_h[:, c:c+1])
            # h-LIF over t; dw-half spikes written into padded tile, gate into sp2
            m_h = mh_p.tile([P, 512], BF16, name=f"m_h{ch}", tag="t")
            sp2 = sp2_p.tile([P, 1024], BF16, name=f"sp2_{ch}", tag="t")
            xp1 = xp1_bufs[ch % 2]
            xh3 = xh.rearrange("p (h q) -> p h q", h=2)
            mh3 = m_h.rearrange("p (h q) -> p h q", h=2)
            xh4 = xh.rearrange("p (a h w) -> p a h w", h=16, w=16)
            xp4 = xp1.rearrange("p (t h w) -> p t h w", t=4, h=18, w=18)
            for t in range(T):
                thr = float(2.0 ** t)
                U3 = xh3[:, :, t*N:(t+1)*N]
                if t > 0:
                    GE.tensor_add(U3, mh3, U3)
                VE.tensor_single_scalar(xp4[:, t, 1:17, 1:17], xh4[:, t], thr, AL.is_ge)
                VE.tensor_single_scalar(sp2[:, t*N:(t+1)*N],
                                        xh[:, 1024 + t*N: 1024 + (t+1)*N], thr, AL.is_ge)
                if t < T - 1:
                    VE.scalar_tensor_tensor(out=mh3, in0=U3, scalar=thr, in1=U3,
                                            op0=AL.is_lt, op1=AL.mult)
            # depthwise conv on the PE: diag(bias) @ ones + sum diag(tap) @ shifted
            acc = acc_p.tile([P, 1024], BF16, name=f"acc{ch}", tag="t")
            for tb in range(2):
                pa = ps_dw.tile([P, 512], F32, name=f"dwa{ch}{tb}", tag="pdw")
                for j, (dy, dx) in enumerate(TAPS):
                    rhs = xp4[:, tb*2:(tb+1)*2, dy:dy+16, dx:dx+16]
                    nc.tensor.matmul(pa, dwd[:, (ch*9+j)*P:(ch*9+j+1)*P], rhs,
                                     start=(j == 0), stop=(j == 8))
                nc.scalar.activation(acc[:, tb*512:(tb+1)*512], pa, AF.Identity,
                                     bias=bdw[:, ch:ch+1], scale=1.0)
            # dw-LIF + gate -> mg0 (t-pair 0) / mg1 (t-pair 1), bf16
            m_dw = mdw_p.tile([P, N], BF16, name=f"m_dw{ch}", tag="t")
            mg0 = mg0_p.tile([P, 512], BF16, name=f"mg0_{ch}", tag="t")
            mg1 = mg1_p.tile([P, 512], BF16, name=f"mg1_{ch}", tag="t")
            for t in range(T):
                thr = float(2.0 ** t)
                mg = mg0 if t < 2 else mg1
                U = acc[:, t*N:(t+1)*N]
                if t > 0:
                    VE.scalar_tensor_tensor(out=U, in0=U, scalar=thr, in1=m_dw,
                                            op0=AL.mult, op1=AL.add)
                VE.scalar_tensor_tensor(out=mg[:, (t % 2)*N:(t % 2 + 1)*N], in0=U,
                                        scalar=thr, in1=sp2[:, t*N:(t+1)*N],
                                        op0=AL.is_ge, op1=AL.mult)
                if t < T - 1:
                    VE.scalar_tensor_tensor(out=m_dw, in0=U, scalar=thr, in1=U,
                                            op0=AL.is_lt, op1=AL.mult)
            mg_t.append(mg1)
            # fc2 t-pair 0 accumulate
            for mt in range(3):
                nc.tensor.matmul(po1[mt], f2_w[ch][:, mt*P:(mt+1)*P],
                                 mg0,
                                 start=(ch == 0), stop=(ch == 7), skip_group_check=True)

        # fc2 t-pair 1 tail
        po2 = [ps_po.tile([P, 512], F32, name=f"po2_{mt}", tag="po")
               for mt in range(3)]
        for mt in range(3):
            for ch in range(8):
                nc.tensor.matmul(po2[mt], f2_w[ch][:, mt*P:(mt+1)*P],
                                 mg_t[ch],
                                 start=(ch == 0), stop=(ch == 7), skip_group_check=True)

        # fc2 evict + final LIF + residual + store
        xo_t = [xo_p.tile([P, 768], BF16, name=f"xo{t}", tag="t") for t in range(T)]
        for t in range(T):
            po = po1 if t < 2 else po2
            for mt in range(3):
                c = mt * 4 + t
                nc.scalar.activation(xo_t[t][:, mt*N:(mt+1)*N],
                                     po[mt][:, (t % 2)*N:(t % 2 + 1)*N],
                                     AF.Identity, bias=b_o[:, c:c+1], scale=a_o[:, c:c+1])
        for t in range(T):
            thr = float(2.0 ** t)
            of = of_p.tile([P, 768], F32, name=f"of{t}", tag="t")
            for mt in range(3):
                U = xo_t[t][:, mt*N:(mt+1)*N]
                if t > 0:
                    GE.tensor_add(U, m_o[mt], U)
                if t < T - 1:
                    VE.scalar_tensor_tensor(out=m_o[mt], in0=U, scalar=thr, in1=U,
                                            op0=AL.is_lt, op1=AL.mult)
                VE.scalar_tensor_tensor(
                    out=of[:, mt*N:(mt+1)*N], in0=U, scalar=thr,
                    in1=xs_kt[mt][:, t*N:(t+1)*N], op0=AL.is_ge, op1=AL.add)
                nc.sync.dma_start(out=d['out'][t*C + mt*P: t*C + (mt+1)*P, :],
                                  in_=of[:, mt*N:(mt+1)*N])


def _build():
    nc = bacc.Bacc()
    with tile.TileContext(nc) as tc:
        with tc.tile_pool(name="dram", bufs=1, space="DRAM") as dram:
            def din(name, shape, dt=F32):
                return dram.tile(shape, dt, kind="ExternalInput", name=name,
                                 uniquify=False)
            d = {
                'xin': din('xin', [C, 4 * N]),
                'out': dram.tile([T * C, N], F32, kind="ExternalOutput",
                                 name='out', uniquify=False),
                'q_w': din('q_w', [384, 384], BF16),
                's_q': din('s_q', [ED, 32]),
                'vk_w': din('vk_w', [384, 484], BF16),
                'rb2': din('rb2', [1, 484], BF16),
                'ones': din('ones', [1, 128], BF16),
                'pj_w': din('pj_w', [384, 384], BF16),
                's_po': din('s_po', [128, 48]),
                'f1_w': din('f1_w', [384, 2048], BF16),
                's_h': din('s_h', [128, 128]),
                'dwd': din('dwd', [128, 72 * 128], BF16),
                'b_dw': din('b_dw', [128, 8]),
                'f2_w': din('f2_w', [1024, 384], BF16),
                'ident': din('ident', [128, 128], BF16),
            }
            _body(nc, tc, d)
    nc.finalize()
    return nc


_NC_CACHE = {}


def _get_nc():
    if 'nc' not in _NC_CACHE:
        _NC_CACHE['nc'] = _build()
    return _NC_CACHE['nc']


def _tcols(a):
    rows, k = a.shape
    out = np.empty((rows, k * 4), np.float32)
    for u in range(k):
        for t in range(4):
            out[:, u * 4 + t] = a[:, u] * (2.0 ** t)
    return out


def _prep_common(inputs):
    inp = {k: np.asarray(v, np.float32) for k, v in inputs.items()}
    bf16 = ml_dtypes.bfloat16

    exp_wT = np.concatenate([inp['exp_w'][e].T for e in range(NE)], axis=1)
    a_q = np.zeros((ED, NE), np.float32)
    b_q = np.zeros((ED, NE), np.float32)
    for e in range(NE):
        a_q[:, e] = 0.5 * inp['exp_g'][e] * S
        b_q[:, e] = 0.5 * inp['exp_b'][e]
    rw = inp['router_w'].T * (inp['router_g'] * S)[None, :]
    vk = np.concatenate([inp['v_w'].T, inp['k_w'].T, rw], axis=1)
    rb2 = np.zeros((1, 484), np.float32)
    rb2[0, 480:484] = (inp['router_b'] * inp['router_g'] * S + inp['router_be'])

    g = inp['dw_g']
    taps = (inp['dw_w'][:, 0] * (0.5 * g * S)[:, None, None]).reshape(HH, 9)
    bias = 0.5 * (inp['dw_b'] * g * S + inp['dw_be'])
    dwd = np.zeros((P, 72, P), np.float32)
    pi = np.arange(P)
    for ch in range(8):
        cg = ch * P + pi
        for j in range(9):
            dwd[pi, ch*9+j, pi] = taps[cg, j]

    com = {
        'q_w': exp_wT.astype(bf16),
        's_q': np.concatenate([_tcols(a_q), _tcols(b_q)], axis=1),
        'vk_w': vk.astype(bf16),
        'rb2': rb2.astype(bf16),
        'ones': np.ones((1, 128), bf16),
        'pj_w': inp['proj_w'].T.astype(bf16),
        's_po': np.concatenate([
            _tcols((0.5 * inp['proj_g'] * S).reshape(3, 128).T),
            _tcols((0.5 * (inp['proj_b'] * inp['proj_g'] * S
                           + inp['proj_be'])).reshape(3, 128).T),
            _tcols((0.5 * inp['fc2_g'] * S).reshape(3, 128).T),
            _tcols((0.5 * (inp['fc2_b'] * inp['fc2_g'] * S
                           + inp['fc2_be'])).reshape(3, 128).T)], axis=1),
        'f1_w': inp['fc1_w'].T.astype(bf16),
        's_h': np.concatenate([
            _tcols((0.5 * inp['fc1_g'] * S).reshape(16, 128).T),
            _tcols((0.5 * (inp['fc1_b'] * inp['fc1_g'] * S
                           + inp['fc1_be'])).reshape(16, 128).T)], axis=1),
        'dwd': dwd.reshape(P, 72 * P).astype(bf16),
        'b_dw': np.ascontiguousarray(bias.reshape(8, P).T),
        'f2_w': inp['fc2_w'].T.astype(bf16),
        'ident': np.eye(128, dtype=bf16),
    }
    return {k: np.ascontiguousarray(v) for k, v in com.items()}


def run(inputs, trace=False, tmpdir=None):
    com = _prep_common(inputs)
    x = np.asarray(inputs['x'], np.float32).reshape(T, B, C, N)
    in_maps = []
    for b in range(B):
        m = dict(com)
        m['xin'] = np.ascontiguousarray(x[:, b].transpose(1, 0, 2).reshape(C, T * N))
        in_maps.append(m)
    res = run_bass_kernel_spmd(_get_nc(), in_maps, list(range(B)),
                               trace=trace, tmpdir=tmpdir)
    out = np.empty((T, B, C, N), np.float32)
    for b in range(B):
        out[:, b] = res.results[b]['out'].reshape(T, C, N)
    return out.reshape(T * B, C, 16, 16), res.exec_time_ns


def kernel(**inputs):
    out, _ = run(inputs)
    return out


# revision 23
# speedup vs baseline: 1.1277x; 1.1277x over previous
"""Trainium2 Bass kernel for nn_Block_31954556682442 (spiking MoE-SSA block).

Sharding: pure data-parallel over batch B=8 -> one sample (4 LIF time steps)
per NeuronCore, zero collectives. v3 design:
  - ALL weight matmuls single-term bf16 (W and x both bf16-rounded); CPU-sim
    rel err 4.4e-3 vs the 2e-2 gate
  - kv-first attention: res_e = q_e^T (k v^T); kv integers <=256 exact bf16;
    k^T computed in v's layout by widening the v matmul rhs (no transposes)
  - attention-path LIF state kept in bf16 (values quantized to 0.5 and
    compare-safe), enabling 2x/4x DVE modes
  - depthwise 3x3 conv moved to the PE: per-(ch,tap) diagonal bf16 weights
    x zero-padded spike tiles; bias via diag @ ones
  - fc2 split: t-pair 0 accumulated across the ch loop (3 PSUM banks),
    t-pair 1 as a tail overlapped with the final LIF
  - LIF scans in 2^t-scaled form as in v2
Self-contained: hardcodes all shapes; no sibling imports.
"""
import numpy as np
import ml_dtypes

import concourse.bacc as bacc
import concourse.mybir as mybir
import concourse.tile as tile
from concourse.bass_utils import run_bass_kernel_spmd

F32 = mybir.dt.float32
BF16 = mybir.dt.bfloat16
AL = mybir.AluOpType
AF = mybir.ActivationFunctionType

T, B, C, N = 4, 8, 384, 256
ED = 96
NE = 4
HID, HH = 2048, 1024
S = float(1.0 / np.sqrt(1.0 + 1e-5))
P = 128


def _body(nc, tc, d):
    from contextlib import ExitStack
    VE = nc.vector
    GE = nc.gpsimd

    with ExitStack() as ctx:
        def pool(name, bufs, space="SBUF"):
            return ctx.enter_context(tc.tile_pool(name=name, bufs=bufs, space=space))

        wp = pool("wp", 1)
        mp = pool("mp", 1)
        xs_p = pool("xs_p", 3)       # (128,1024) f32 x, doubles as x_new
        xq_p = pool("xq_p", 3)       # (128,1024) bf16 rounded x
        xqt_p = pool("xqt_p", 3)     # (96,1024) f32 q pre-act
        xvt_p = pool("xvt_p", 3)     # (128,960) f32 v|k pre-act
        xrt_p = pool("xrt_p", 2)     # (128,8)
        spq_p = pool("spq_p", 4)     # (96,1024) bf16 q spikes
        vks_p = pool("vks_p", 4)     # (128,960) bf16 v|k spikes
        wsp_p = pool("wsp_p", 4)     # (128,8) f32 router spikes
        kv_p = pool("kv_p", 2)       # (96,384) bf16
        xr_p = pool("xr_p", 4)       # (128,768) bf16 res pre-act
        rs_p = pool("rs_p", 2)       # (128,768) bf16 masked res spikes
        y_p = pool("y_p", 4)         # (128,768) bf16
        ydn_p = pool("ydn_p", 3)     # (128,1024) bf16
        xp_p = pool("xp_p", 2)       # (128,768) f32 proj pre-act
        xh_p = pool("xh_p", 3)       # (128,2048) f32
        sp2_p = pool("sp2_p", 3)     # (128,1024) bf16 gate spikes
        xp1_p = pool("xp1_p", 1)     # padded dw spikes, 2 tiles
        acc_p = pool("acc_p", 3)     # (128,1024) f32 dw-conv acc
        mg0_p = pool("mg0_p", 2)     # (128,512) bf16 t-pair0 gated spikes
        mg1_p = pool("mg1_p", 8)     # (128,512) bf16 t-pair1 gated spikes
        mh_p = pool("mh_p", 2)       # (128,512) f32
        mdw_p = pool("mdw_p", 2)     # (128,256) f32
        xo_p = pool("xo_p", 2)       # (128,768) bf16 fc2 pre-act
        of_p = pool("of_p", 2)       # (128,768) f32 final out

        psum_ad = ExitStack()
        ps_ad = psum_ad.enter_context(
            tc.tile_pool(name="ps_ad", bufs=6, space="PSUM"))

        # ---------------- weight loads ----------------
        def wload(name, shape, dt=F32, src=None):
            w = wp.tile(shape, dt, name=name, tag=name)
            nc.sync.dma_start(out=w, in_=d[name] if src is None else src)
            return w

        ident = wload('ident', [P, P], BF16)
        # PE warmup: dummy matmuls to flip HAM to K=8/8 before phase A
        pwarm = ps_ad.tile([P, P], F32, name="pwarm", tag="pm")
        for wi in range(12):
            nc.tensor.matmul(pwarm, ident, ident, start=True, stop=True)
        warm_sink = wp.tile([P, 1], F32, name="warm_sink", tag="warm_sink")
        nc.scalar.activation(warm_sink, pwarm[:, 0:1], AF.Copy)

        # xs first (phase A starts on these)
        xs_kt = []
        for kt in range(3):
            x_ = xs_p.tile([P, 4 * N], F32, name=f"xs{kt}", tag="t")
            xs_kt.append(x_)
        for kt in range(3):
            nc.sync.dma_start(out=xs_kt[kt], in_=d['xin'][kt*P:(kt+1)*P, :])
        q_w, vk_w, pj_w, f1_w, f2_w = [], [], [], [], []
        for kt in range(3):
            q_w.append(wload(f'q_w{kt}', [P, 384], BF16, d['q_w'][kt*P:(kt+1)*P, :]))
            vk_w.append(wload(f'vk_w{kt}', [P, 484], BF16, d['vk_w'][kt*P:(kt+1)*P, :]))
        sq = wload('s_q', [ED, 32])
        a_q, b_q = sq[:, 0:16], sq[:, 16:32]
        rb2 = wload('rb2', [1, 484], BF16); ones = wload('ones', [1, P], BF16)
        for kt in range(3):
            pj_w.append(wload(f'pj_w{kt}', [P, 384], BF16, d['pj_w'][kt*P:(kt+1)*P, :]))
        spo = wload('s_po', [P, 48])
        a_p, b_p = spo[:, 0:12], spo[:, 12:24]
        a_o, b_o = spo[:, 24:36], spo[:, 36:48]
        for kt in range(3):
            f1_w.append(wload(f'f1_w{kt}', [P, 2048], BF16, d['f1_w'][kt*P:(kt+1)*P, :]))
        sh = wload('s_h', [P, 128])
        a_h, b_h = sh[:, 0:64], sh[:, 64:128]
        dwd = wload('dwd', [P, 72 * P], BF16)
        bdw = wload('b_dw', [P, 8])
        for ch in range(8):
            f2_w.append(wload(f'f2_w{ch}', [P, 384], BF16, d['f2_w'][ch*P:(ch+1)*P, :]))

        # ---------------- x -> bf16 ----------------
        xq_kt = []
        for kt in range(3):
            q_ = xq_p.tile([P, 4 * N], BF16, name=f"xq{kt}", tag="t")
            nc.scalar.activation(q_, xs_kt[kt], AF.Copy)
            xq_kt.append(q_)

        # ---------------- phase A: q / v|k / router matmuls + evicts ----------------
        m_kq = mp.tile([ED, 4 * N], BF16, name="m_kq", tag="m_kq")
        m_vt = mp.tile([P, 968], BF16, name="m_vt", tag="m_vt")
        m_p = mp.tile([P, 768], BF16, name="m_p", tag="m_p")
        m_o = [mp.tile([P, N], BF16, name=f"m_o{i}", tag=f"m_o{i}") for i in range(3)]

        xq_t = [xqt_p.tile([ED, 4 * N], BF16, name=f"xqt{t}", tag="t") for t in range(T)]
        xvt_t = [xvt_p.tile([P, 968], BF16, name=f"xvt{t}", tag="t") for t in range(T)]

        for tp in range(2):
            for u in range(NE):
                pt = ps_ad.tile([ED, 512], F32, name=f"pq{u}_{tp}", tag="pm")
                for kt in range(3):
                    nc.tensor.matmul(pt, q_w[kt][:, 96*u:96*(u+1)],
                                     xq_kt[kt][:, tp*512:(tp+1)*512],
                                     start=(kt == 0), stop=(kt == 2))
                for ti in range(2):
                    t = tp * 2 + ti
                    c = u * 4 + t
                    nc.scalar.activation(xq_t[t][:, u*N:(u+1)*N], pt[:, ti*N:(ti+1)*N],
                                         AF.Identity, bias=b_q[:, c:c+1], scale=a_q[:, c:c+1])
        for t in range(T):
            for mt in range(2):
                pv = ps_ad.tile([P, 484], F32, name=f"pvt{t}_{mt}", tag="pm")
                for kt in range(3):
                    nc.tensor.matmul(pv, xq_kt[kt][:, t*N + mt*P: t*N + (mt+1)*P],
                                     vk_w[kt], start=(kt == 0), stop=False,
                                     skip_group_check=True)
                nc.tensor.matmul(pv[:, 480:484], ones, rb2[:, 480:484],
                                 start=False, stop=True, skip_group_check=True)
                nc.scalar.activation(xvt_t[t][:, mt*484:(mt+1)*484], pv, AF.Copy,
                                     bias=0.0, scale=0.5 * float(2.0 ** t))

        # ---------------- phase B: LIF scans for q / v|k / r ----------------
        sp_q, vk_sp, w_sp = [], [], []
        for t in range(T):
            thr = float(2.0 ** t)
            U = xq_t[t]
            if t > 0:
                VE.tensor_add(U, m_kq, U)
            sp = spq_p.tile([ED, 4 * N], BF16, name=f"spq{t}", tag="t")
            VE.tensor_single_scalar(sp, U, thr, AL.is_ge)
            if t < T - 1:
                VE.scalar_tensor_tensor(out=m_kq, in0=U, scalar=thr, in1=U,
                                        op0=AL.is_lt, op1=AL.mult)
            sp_q.append(sp)

            U = xvt_t[t]
            if t > 0:
                VE.tensor_add(U, m_vt, U)
            vs = vks_p.tile([P, 968], BF16, name=f"vks{t}", tag="t")
            VE.tensor_single_scalar(vs, U, thr, AL.is_ge)
            # router spike columns as f32 (tensor_scalar scalar2 operand)
            ws = wsp_p.tile([P, 8], F32, name=f"wsp{t}", tag="t")
            U3 = U.rearrange("p (m c) -> p m c", m=2)
            VE.tensor_single_scalar(ws.rearrange("p (m c) -> p m c", m=2),
                                    U3[:, :, 480:484], thr, AL.is_ge)
            w_sp.append(ws)
            if t < T - 1:
                VE.scalar_tensor_tensor(out=m_vt, in0=U, scalar=thr, in1=U,
                                        op0=AL.is_lt, op1=AL.mult)
            vk_sp.append(vs)
            # keep-warm: tiny matmul chained on this step's spikes so the PE
            # HAM window never sees a fully idle interval during the LIF scan
            nc.tensor.matmul(pwarm[:, 0:1], ident, vs[:, 0:1], start=True, stop=True)

        # ---------------- phase C: kv-first experts ----------------
        # masked spikes: w_e*(U>=thr) via one tensor_scalar (4x mode);
        # y accumulated with plain bf16 tensor_tensor adds (2x mode)
        y = [None] * T
        m_res_e = [mp.tile([P, 768], BF16, name=f"m_res{e}", tag=f"m_res{e}")
                   for e in range(NE)]
        for t in range(T):
            thr = float(2.0 ** t)
            pkv = ps_ad.tile([ED, 384], F32, name=f"pkv{t}", tag="pm")
            for mt in range(2):
                nc.tensor.matmul(pkv, vk_sp[t][:, mt*484+384:mt*484+480],
                                 vk_sp[t][:, mt*484:mt*484+384],
                                 start=(mt == 0), stop=(mt == 1))
            kv = kv_p.tile([ED, 384], BF16, name=f"kv{t}", tag="t")
            nc.scalar.activation(kv, pkv, AF.Copy, bias=0.0, scale=0.5 * thr)
            yt = y_p.tile([P, 768], BF16, name=f"y{t}", tag="t")
            y[t] = yt
            for e in range(NE):
                m_res = m_res_e[e]
                xr = xr_p.tile([P, 768], BF16, name=f"xres{e}{t}", tag="t")
                for mt in range(2):
                    pr_ = ps_ad.tile([P, 384], F32, name=f"pres{e}{t}{mt}", tag="pm")
                    nc.tensor.matmul(pr_, sp_q[t][:, e*N + mt*P: e*N + (mt+1)*P],
                                     kv, start=True, stop=True)
                    nc.scalar.activation(xr[:, mt*384:(mt+1)*384], pr_, AF.Copy)
                U = xr
                if t > 0:
                    VE.tensor_add(U, m_res, U)
                dst = yt if e == 0 else rs_p.tile([P, 768], BF16,
                                                  name=f"rsm{e}{t}", tag="t")
                for mt in range(2):
                    VE.tensor_scalar(out=dst[:, mt*384:(mt+1)*384],
                                     in0=U[:, mt*384:(mt+1)*384],
                                     scalar1=thr,
                                     scalar2=w_sp[t][:, mt*4+e:mt*4+e+1],
                                     op0=AL.is_ge, op1=AL.mult)
                if t < T - 1:
                    VE.scalar_tensor_tensor(out=m_res, in0=U, scalar=thr, in1=U,
                                            op0=AL.is_lt, op1=AL.mult)
                if e > 0:
                    VE.tensor_add(yt, yt, dst)
                nc.tensor.matmul(pwarm[:, 0:1], ident, U[:, 0:1], start=True, stop=True)

        # ---------------- phase D: transpose y, proj, LIF, residual ----------------
        ydn = [ydn_p.tile([P, 4 * N], BF16, name=f"ydn{dt}", tag="t") for dt in range(3)]
        xp_t = [xp_p.tile([P, 768], BF16, name=f"xp{t}", tag="t") for t in range(T)]
        for tp in range(2):
            for t in (tp * 2, tp * 2 + 1):
                for mt in range(2):
                    for dt in range(3):
                        ptr = ps_ad.tile([P, P], BF16, name=f"ptr{t}{mt}{dt}", tag="pm")
                        nc.tensor.transpose(
                            ptr, y[t][:, mt*384 + dt*P: mt*384 + (dt+1)*P], ident)
                        nc.scalar.activation(ydn[dt][:, t*N + mt*P: t*N + (mt+1)*P],
                                             ptr, AF.Copy)
            for mt in range(3):
                pp = ps_ad.tile([P, 512], F32, name=f"pp{mt}_{tp}", tag="pm")
                for kt in range(3):
                    nc.tensor.matmul(pp, pj_w[kt][:, mt*P:(mt+1)*P],
                                     ydn[kt][:, tp*512:(tp+1)*512],
                                     start=(kt == 0), stop=(kt == 2))
                for ti in range(2):
                    t = tp * 2 + ti
                    c = mt * 4 + t
                    nc.scalar.activation(xp_t[t][:, mt*N:(mt+1)*N], pp[:, ti*N:(ti+1)*N],
                                         AF.Identity, bias=b_p[:, c:c+1], scale=a_p[:, c:c+1])
            for t in (tp * 2, tp * 2 + 1):
                thr = float(2.0 ** t)
                U = xp_t[t]
                if t > 0:
                    VE.tensor_add(U, m_p, U)
                if t < T - 1:
                    VE.scalar_tensor_tensor(out=m_p, in0=U, scalar=thr, in1=U,
                                            op0=AL.is_lt, op1=AL.mult)
                for mt in range(3):
                    # x_new overwrites xs in place (residual add)
                    VE.scalar_tensor_tensor(
                        out=xs_kt[mt][:, t*N:(t+1)*N], in0=U[:, mt*N:(mt+1)*N],
                        scalar=thr, in1=xs_kt[mt][:, t*N:(t+1)*N],
                        op0=AL.is_ge, op1=AL.add)
            # x_new -> bf16 for this time-pair (fc1 can start on tp=0)
            for kt in range(3):
                nc.scalar.activation(xq_kt[kt][:, tp*512:(tp+1)*512],
                                     xs_kt[kt][:, tp*512:(tp+1)*512], AF.Copy)

        # ---------------- phase E: MLP ----------------
        psum_ad.close()
        ps_e = ctx.enter_context(tc.tile_pool(name="ps_e", bufs=3, space="PSUM"))
        ps_dw = ctx.enter_context(tc.tile_pool(name="ps_dw", bufs=2, space="PSUM"))
        ps_po = ctx.enter_context(tc.tile_pool(name="ps_po", bufs=3, space="PSUM"))
        # padded dw-spike tiles (zero borders written once)
        xp1_bufs = [xp1_p.tile([P, 1296], BF16, name=f"xp1_{i}", tag=f"xp1_{i}")
                    for i in range(2)]
        for b_ in xp1_bufs:
            GE.memset(b_, 0.0)
        po1 = [ps_po.tile([P, 512], F32, name=f"po1_{mt}", tag="po")
               for mt in range(3)]
        TAPS = [(dy, dx) for dy in range(3) for dx in range(3)]
        mg_t = []
        for ch in range(8):
            xh = xh_p.tile([P, 2048], BF16, name=f"xh{ch}", tag="t")
            for half in range(2):
                mth = ch + 8 * half
                for tp in range(2):
                    ph = ps_e.tile([P, 512], F32, name=f"ph{ch}{half}{tp}", tag="pm")
                    for kt in range(3):
                        nc.tensor.matmul(ph, f1_w[kt][:, mth*P:(mth+1)*P],
                                         xq_kt[kt][:, tp*512:(tp+1)*512],
                                         start=(kt == 0), stop=(kt == 2))
                    for ti in range(2):
                        t = tp * 2 + ti
                        c = mth * 4 + t
                        nc.scalar.activation(
                            xh[:, half*1024 + t*N: half*1024 + (t+1)*N],
                            ph[:, ti*N:(ti+1)*N], AF.Identity,
                            bias=b_h[:, c:c+1], scale=a_h[:, c:c+1])
            # h-LIF over t; dw-half spikes written into padded tile, gate into sp2
            m_h = mh_p.tile([P, 512], BF16, name=f"m_h{ch}", tag="t")
            sp2 = sp2_p.tile([P, 1024], BF16, name=f"sp2_{ch}", tag="t")
            xp1 = xp1_bufs[ch % 2]
            xh3 = xh.rearrange("p (h q) -> p h q", h=2)
            mh3 = m_h.rearrange("p (h q) -> p h q", h=2)
            xh4 = xh.rearrange("p (a h w) -> p a h w", h=16, w=16)
            xp4 = xp1.rearrange("p (t h w) -> p t h w", t=4, h=18, w=18)
            for t in range(T):
                thr = float(2.0 ** t)
                U3 = xh3[:, :, t*N:(t+1)*N]
                if t > 0:
                    VE.tensor_add(U3, mh3, U3)
                VE.tensor_single_scalar(xp4[:, t, 1:17, 1:17], xh4[:, t], thr, AL.is_ge)
                VE.tensor_single_scalar(sp2[:, t*N:(t+1)*N],
                                        xh[:, 1024 + t*N: 1024 + (t+1)*N], thr, AL.is_ge)
                if t < T - 1:
                    VE.scalar_tensor_tensor(out=mh3, in0=U3, scalar=thr, in1=U3,
                                            op0=AL.is_lt, op1=AL.mult)
            # depthwise conv on the PE: diag(bias) @ ones + sum diag(tap) @ shifted
            acc = acc_p.tile([P, 1024], BF16, name=f"acc{ch}", tag="t")
            for tb in range(2):
                pa = ps_dw.tile([P, 512], F32, name=f"dwa{ch}{tb}", tag="pdw")
                for j, (dy, dx) in enumerate(TAPS):
                    rhs = xp4[:, tb*2:(tb+1)*2, dy:dy+16, dx:dx+16]
                    nc.tensor.matmul(pa, dwd[:, (ch*9+j)*P:(ch*9+j+1)*P], rhs,
                                     start=(j == 0), stop=(j == 8))
                nc.scalar.activation(acc[:, tb*512:(tb+1)*512], pa, AF.Identity,
                                     bias=bdw[:, ch:ch+1], scale=1.0)
            # dw-LIF + gate -> mg0 (t-pair 0) / mg1 (t-pair 1), bf16
            m_dw = mdw_p.tile([P, N], BF16, name=f"m_dw{ch}", tag="t")
            mg0 = mg0_p.tile([P, 512], BF16, name=f"mg0_{ch}", tag="t")
            mg1 = mg1_p.tile([P, 512], BF16, name=f"mg1_{ch}", tag="t")
            for t in range(T):
                thr = float(2.0 ** t)
                mg = mg0 if t < 2 else mg1
                U = acc[:, t*N:(t+1)*N]
                if t > 0:
                    VE.scalar_tensor_tensor(out=U, in0=U, scalar=thr, in1=m_dw,
                                            op0=AL.mult, op1=AL.add)
                VE.scalar_tensor_tensor(out=mg[:, (t % 2)*N:(t % 2 + 1)*N], in0=U,
                                        scalar=thr, in1=sp2[:, t*N:(t+1)*N],
                                        op0=AL.is_ge, op1=AL.mult)
                if t < T - 1:
                    VE.scalar_tensor_tensor(out=m_dw, in0=U, scalar=thr, in1=U,
                                            op0=AL.is_lt, op1=AL.mult)
            mg_t.append(mg1)
            # fc2 t-pair 0 accumulate
            for mt in range(3):
                nc.tensor.matmul(po1[mt], f2_w[ch][:, mt*P:(mt+1)*P],
                                 mg0,
                                 start=(ch == 0), stop=(ch == 7), skip_group_check=True)

        # fc2 t-pair 1 tail
        po2 = [ps_po.tile([P, 512], F32, name=f"po2_{mt}", tag="po")
               for mt in range(3)]
        for mt in range(3):
            for ch in range(8):
                nc.tensor.matmul(po2[mt], f2_w[ch][:, mt*P:(mt+1)*P],
                                 mg_t[ch],
                                 start=(ch == 0), stop=(ch == 7), skip_group_check=True)

        # fc2 evict + final LIF + residual + store
        xo_t = [xo_p.tile([P, 768], BF16, name=f"xo{t}", tag="t") for t in range(T)]
        for t in range(T):
            po = po1 if t < 2 else po2
            for mt in range(3):
                c = mt * 4 + t
                nc.scalar.activation(xo_t[t][:, mt*N:(mt+1)*N],
                                     po[mt][:, (t % 2)*N:(t % 2 + 1)*N],
                                     AF.Identity, bias=b_o[:, c:c+1], scale=a_o[:, c:c+1])
        for t in range(T):
            thr = float(2.0 ** t)
            of = of_p.tile([P, 768], F32, name=f"of{t}", tag="t")
            for mt in range(3):
                U = xo_t[t][:, mt*N:(mt+1)*N]
                if t > 0:
                    GE.tensor_add(U, m_o[mt], U)
                if t < T - 1:
                    VE.scalar_tensor_tensor(out=m_o[mt], in0=U, scalar=thr, in1=U,
                                            op0=AL.is_lt, op1=AL.mult)
                VE.scalar_tensor_tensor(
                    out=of[:, mt*N:(mt+1)*N], in0=U, scalar=thr,
                    in1=xs_kt[mt][:, t*N:(t+1)*N], op0=AL.is_ge, op1=AL.add)
                nc.sync.dma_start(out=d['out'][t*C + mt*P: t*C + (mt+1)*P, :],
                                  in_=of[:, mt*N:(mt+1)*N])


def _build():
    nc = bacc.Bacc()
    with tile.TileContext(nc) as tc:
        with tc.tile_pool(name="dram", bufs=1, space="DRAM") as dram:
            def din(name, shape, dt=F32):
                return dram.tile(shape, dt, kind="ExternalInput", name=name,
                                 uniquify=False)
            d = {
                'xin': din('xin', [C, 4 * N]),
                'out': dram.tile([T * C, N], F32, kind="ExternalOutput",
                                 name='out', uniquify=False),
                'q_w': din('q_w', [384, 384], BF16),
                's_q': din('s_q', [ED, 32]),
                'vk_w': din('vk_w', [384, 484], BF16),
                'rb2': din('rb2', [1, 484], BF16),
                'ones': din('ones', [1, 128], BF16),
                'pj_w': din('pj_w', [384, 384], BF16),
                's_po': din('s_po', [128, 48]),
                'f1_w': din('f1_w', [384, 2048], BF16),
                's_h': din('s_h', [128, 128]),
                'dwd': din('dwd', [128, 72 * 128], BF16),
                'b_dw': din('b_dw', [128, 8]),
                'f2_w': din('f2_w', [1024, 384], BF16),
                'ident': din('ident', [128, 128], BF16),
            }
            _body(nc, tc, d)
    nc.finalize()
    return nc


_NC_CACHE = {}


def _get_nc():
    if 'nc' not in _NC_CACHE:
        _NC_CACHE['nc'] = _build()
    return _NC_CACHE['nc']


def _tcols(a):
    rows, k = a.shape
    out = np.empty((rows, k * 4), np.float32)
    for u in range(k):
        for t in range(4):
            out[:, u * 4 + t] = a[:, u] * (2.0 ** t)
    return out


def _prep_common(inputs):
    inp = {k: np.asarray(v, np.float32) for k, v in inputs.items()}
    bf16 = ml_dtypes.bfloat16

    exp_wT = np.concatenate([inp['exp_w'][e].T for e in range(NE)], axis=1)
    a_q = np.zeros((ED, NE), np.float32)
    b_q = np.zeros((ED, NE), np.float32)
    for e in range(NE):
        a_q[:, e] = 0.5 * inp['exp_g'][e] * S
        b_q[:, e] = 0.5 * inp['exp_b'][e]
    rw = inp['router_w'].T * (inp['router_g'] * S)[None, :]
    vk = np.concatenate([inp['v_w'].T, inp['k_w'].T, rw], axis=1)
    rb2 = np.zeros((1, 484), np.float32)
    rb2[0, 480:484] = (inp['router_b'] * inp['router_g'] * S + inp['router_be'])

    g = inp['dw_g']
    taps = (inp['dw_w'][:, 0] * (0.5 * g * S)[:, None, None]).reshape(HH, 9)
    bias = 0.5 * (inp['dw_b'] * g * S + inp['dw_be'])
    dwd = np.zeros((P, 72, P), np.float32)
    pi = np.arange(P)
    for ch in range(8):
        cg = ch * P + pi
        for j in range(9):
            dwd[pi, ch*9+j, pi] = taps[cg, j]

    com = {
        'q_w': exp_wT.astype(bf16),
        's_q': np.concatenate([_tcols(a_q), _tcols(b_q)], axis=1),
        'vk_w': vk.astype(bf16),
        'rb2': rb2.astype(bf16),
        'ones': np.ones((1, 128), bf16),
        'pj_w': inp['proj_w'].T.astype(bf16),
        's_po': np.concatenate([
            _tcols((0.5 * inp['proj_g'] * S).reshape(3, 128).T),
            _tcols((0.5 * (inp['proj_b'] * inp['proj_g'] * S
                           + inp['proj_be'])).reshape(3, 128).T),
            _tcols((0.5 * inp['fc2_g'] * S).reshape(3, 128).T),
            _tcols((0.5 * (inp['fc2_b'] * inp['fc2_g'] * S
                           + inp['fc2_be'])).reshape(3, 128).T)], axis=1),
        'f1_w': inp['fc1_w'].T.astype(bf16),
        's_h': np.concatenate([
            _tcols((0.5 * inp['fc1_g'] * S).reshape(16, 128).T),
            _tcols((0.5 * (inp['fc1_b'] * inp['fc1_g'] * S
                           + inp['fc1_be'])).reshape(16, 128).T)], axis=1),
        'dwd': dwd.reshape(P, 72 * P).astype(bf16),
        'b_dw': np.ascontiguousarray(bias.reshape(8, P).T),
        'f2_w': inp['fc2_w'].T.astype(bf16),
        'ident': np.eye(128, dtype=bf16),
    }
    return {k: np.ascontiguousarray(v) for k, v in com.items()}


def run(inputs, trace=False, tmpdir=None):
    com = _prep_common(inputs)
    x = np.asarray(inputs['x'], np.float32).reshape(T, B, C, N)
    in_maps = []
    for b in range(B):
        m = dict(com)
        m['xin'] = np.ascontiguousarray(x[:, b].transpose(1, 0, 2).reshape(C, T * N))
        in_maps.append(m)
    res = run_bass_kernel_spmd(_get_nc(), in_maps, list(range(B)),
                               trace=trace, tmpdir=tmpdir)
    out = np.empty((T, B, C, N), np.float32)
    for b in range(B):
        out[:, b] = res.results[b]['out'].reshape(T, C, N)
    return out.reshape(T * B, C, 16, 16), res.exec_time_ns


def kernel(**inputs):
    out, _ = run(inputs)
    return out
